# revision 12
# baseline (speedup 1.0000x reference)
"""AttentionBlock (GroupNorm -> qkv conv1x1 -> 4-head attention over L=4096
-> proj conv1x1 -> residual) on 8 Trainium2 NeuronCores.

Sharding: one (batch, head) pair per core (2 batches x 4 heads = 8 cores).
head_dim = 128 = partition width.

v3 design (on top of the v2 fp8 DoubleRow + split-exp design):
  - exp is emitted at PAIR granularity: each ep's two score e-tiles land in
    one [128, 2, 512] f32 PSUM tile (2 banks) and a SINGLE 1024-elem
    instruction exps the pair -- ScalarE (true Exp) or DVE (Schraudolph),
    alternating 9:7 per chunk.  Halves the per-instruction PSUM-access and
    sequencer overhead on both engines.
  - k production (chunks 2..7) is interleaved into chunk 0's ep loop so the
    PE never sits idle waiting on the serial phase-D chain; vT e-tiles are
    produced 4-at-a-time into one PSUM bank and evacuated with a single
    512-elem cast, alternating ScalarE/DVE.
  - x loads in 8 transfers with 2KB per-partition lines spread over 4 DMA
    queues; weight staging + big memsets moved to the idle Pool sequencer.
  - GroupNorm weight fold split across ScalarE/DVE; final y store split
    over 4 queues.
  - The k bias (and its GroupNorm correction) is dropped entirely: adding a
    constant to every key shifts each score column by a constant along the
    softmax axis, so softmax is invariant to it.
  - GroupNorm stats: sum(x) per group via fp8 DoubleRow indicator matmuls;
    sum(x^2) via bf16 x squared on ACT/DVE with accum_out.
  - GroupNorm affine folded into the fp8 qkv weights; B exported for the
    host-side v correction; host divides by Z and adds residual.
"""

import math
import os
import sys

import numpy as np
import ml_dtypes

if "/opt/trn_rl_repo" not in sys.path:
    sys.path.insert(0, "/opt/trn_rl_repo")

C = 512
L = 4096
NH = 4
HD = 128
NGROUPS = 32
GSIZE = C // NGROUPS  # 16
EPS = 1e-5
NCORES = 8
NB = 2
DC = 512          # d-chunk width for attention
NDC = L // DC     # 8
NET = L // 128    # 32 e-tiles
BF16 = ml_dtypes.bfloat16
FP8 = ml_dtypes.float8_e4m3

# Schraudolph constants: fp8e4 bits ~= round(8*(log2(v)+7)) for v=exp(s*scale)
SCALE = 1.0 / math.sqrt(HD)
A_SCH = 8.0 / math.log(2.0) * SCALE
B_SCH = 56.0 - 0.3435

# per-chunk exp engine assignment: True -> ScalarE (true Exp),
# False -> DVE (Schraudolph).  9 ACT : 7 DVE.
ACT_SET = frozenset({0, 2, 4, 6, 8, 10, 12, 14, 15})

_DMA_INSTS = ("InstDMACopy", "InstDMATranspose", "InstCollectiveCompute")


def _split_multi_sync(nc, mybir):
    """This walrus build encodes at most one sync wait and one sync update
    per instruction.  Move extra waits onto preceding single-wait NOPs and
    extra updates onto following NOPs (same engine; a following NOP's update
    fires only after the instruction completes for engine-datapath ops)."""
    n_w = n_u = 0
    for fn in nc.m.functions:
        for blk in fn.blocks:
            new = []
            for inst in blk.instructions:
                si = getattr(inst, "sync_info", None)
                pre, post = [], []
                if si is not None and si.on_wait is not None and len(si.on_wait) > 1:
                    waits = list(si.on_wait)
                    for w in waits[:-1]:
                        n_w += 1
                        nop = mybir.InstNoOp(name=f"wsplit-{n_w}", ins=[], outs=[])
                        nop.engine = inst.engine
                        nop.bass_nofuse = True
                        nop.sync_info = mybir.SyncInfo(on_wait=[w], on_update=[])
                        pre.append(nop)
                    si.on_wait[:] = [waits[-1]]
                if si is not None and si.on_update is not None and len(si.on_update) > 1:
                    kind = type(inst).__name__
                    assert kind not in _DMA_INSTS, (
                        f"multi-update on async {kind} cannot be split: {inst.name}"
                    )
                    upds = list(si.on_update)
                    for u in upds[1:]:
                        n_u += 1
                        nop = mybir.InstNoOp(name=f"usplit-{n_u}", ins=[], outs=[])
                        nop.engine = inst.engine
                        nop.bass_nofuse = True
                        nop.sync_info = mybir.SyncInfo(on_wait=[], on_update=[u])
                        post.append(nop)
                    si.on_update[:] = [upds[0]]
                new.extend(pre)
                new.append(inst)
                new.extend(post)
            blk.instructions[:] = new
    return n_w, n_u


_NC = None


def _build_nc(split_sync=True):
    import concourse.bass as bass
    import concourse.tile as tile
    from concourse import mybir
    from concourse.vector_clock import ScopedClock

    def _drain_and_barrier_single(self, tick_clock, wait_clock):
        drain_inst = self.nc.sync.drain()
        wait_clock.add_sem_waits(drain_inst.ins,
                                 ScopedClock({None: tick_clock.global_clock}))
        self.nc.all_engine_barrier()
        popped = self.nc._tile_sem_poison_stack.pop()
        assert popped is self._sem_poison
        self.nc.clear_and_free_semaphores(list(self.sems.allocated().values()))

    tile.TileContext._drain_and_barrier = _drain_and_barrier_single

    f32 = mybir.dt.float32
    bf16 = mybir.dt.bfloat16
    fp8 = mybir.dt.float8e4
    i8 = mybir.dt.int8
    u32 = mybir.dt.uint32
    nc = bass.Bass("TRN2")

    DR = mybir.MatmulPerfMode.DoubleRow

    xb8 = nc.dram_tensor("xb8", [C, L], fp8, kind="ExternalInput")
    # packed: per K-tile kk, columns [wq (128) | wk (128) | wv (128)]
    wqkv = nc.dram_tensor("wqkv", [C, 3 * HD], fp8, kind="ExternalInput")
    wp_t = nc.dram_tensor("wp_t", [HD, C], bf16, kind="ExternalInput")
    bq_d = nc.dram_tensor("bq_d", [HD, 1], f32, kind="ExternalInput")
    gnwb = nc.dram_tensor("gnwb", [C, 2], f32, kind="ExternalInput")
    g_b = nc.dram_tensor("g_b", [NGROUPS, C], f32, kind="ExternalInput")
    gt8_d = nc.dram_tensor("gt8_d", [C, NGROUPS], fp8, kind="ExternalInput")
    gt16_d = nc.dram_tensor("gt16_d", [C, NGROUPS], bf16, kind="ExternalInput")

    yt = nc.dram_tensor("yt", [L, C], bf16, kind="ExternalOutput")
    zz = nc.dram_tensor("zz", [1, L], f32, kind="ExternalOutput")
    b_out = nc.dram_tensor("b_out", [HD, 4], f32, kind="ExternalOutput")

    Exp = mybir.ActivationFunctionType.Exp
    Ln = mybir.ActivationFunctionType.Ln
    Copy = mybir.ActivationFunctionType.Copy
    Alu = mybir.AluOpType

    with tile.TileContext(nc) as tc:
        import contextlib

        with contextlib.ExitStack() as ctx:
            # ---------- pools that live for the whole kernel ----------
            p_xn = ctx.enter_context(tc.tile_pool(name="p_xn", bufs=1))
            p_w = ctx.enter_context(tc.tile_pool(name="p_w", bufs=1))
            p_qkv = ctx.enter_context(tc.tile_pool(name="p_qkv", bufs=1))

            xn8 = p_xn.tile([128, 4, L], fp8, name="xn8")

            # weights / constants
            wqkv_sb = p_w.tile([128, 4, 3 * HD], fp8, name="wqkv_sb")
            wp2 = p_w.tile([128, C], bf16, name="wp2")
            ones2 = p_w.tile([128, 2, 128], fp8, name="ones2")
            warm_sb = p_w.tile([128, 64], bf16, name="warm_sb")
            warm_sb2 = p_w.tile([128, 512], bf16, name="warm_sb2")
            bq_sb = p_w.tile([128, 1], f32, name="bq_sb")
            bq2_sb = p_w.tile([128, 1], f32, name="bq2_sb")
            g_sb = p_w.tile([NGROUPS, C], f32, name="g_sb")
            gt8 = p_w.tile([128, 4, NGROUPS], fp8, name="gt8")
            gt16 = p_w.tile([128, 4, NGROUPS], bf16, name="gt16")
            gnwb_sb = p_w.tile([128, 4, 2], f32, name="gnwb_sb")
            eps_sb = p_w.tile([NGROUPS, 1], f32, name="eps_sb")
            zsave = p_w.tile([1, L], f32, name="zsave")

            # q2: [ 8 chunks x 512 real q | 4096 zeros ]  (fake DoubleRow rhs)
            q2 = p_qkv.tile([128, 2 * L], fp8, name="q2")
            q2v = q2.rearrange("p (j n d) -> p j n d", j=2, d=DC)
            # k: 33 e-tiles of 128 (last one zero padding for the fake pair)
            k8 = p_qkv.tile([128, 33 * 128], fp8, name="k8")
            k8v = k8.rearrange("p (t e) -> p t e", e=128)
            vt8 = p_qkv.tile([128, L], fp8, name="vt8")
            vt8v = vt8.rearrange("p (t e) -> p t e", e=128)
            ou2 = p_qkv.tile([128, 512], bf16, name="ou2")

            def wslice(kk, which):
                return wqkv_sb[:, kk, 128 * which:128 * (which + 1)]

            def wpair(i, which):
                # [128, 2, 128] K-tile pair (2i, 2i+1) of wq/wk/wv
                return wqkv_sb[:, 2 * i:2 * i + 2,
                               128 * which:128 * (which + 1)]

            # PE warmup: keep the PE p-state ramped while x loads
            with tc.tile_pool(name="p_warm", bufs=1, space="PSUM") as p_warm:
                warm_ps = p_warm.tile([64, 512], f32, name="warm_ps")
                nc.gpsimd.memset(warm_sb[:], 0.125)
                nc.gpsimd.memset(warm_sb2[:], 0.125)
                for _ in range(12):
                    nc.tensor.matmul(warm_ps[:], warm_sb[:, 0:64], warm_sb2[:],
                                     start=True, stop=True)

                # weight staging on the (idle) Pool HWDGE queue
                nc.gpsimd.dma_start(gt8[:], gt8_d.rearrange("(t p) g -> p t g", p=128))
                nc.gpsimd.dma_start(gt16[:], gt16_d.rearrange("(t p) g -> p t g", p=128))
                nc.gpsimd.dma_start(gnwb_sb[:], gnwb.rearrange("(t p) o -> p t o", p=128))
                nc.gpsimd.dma_start(g_sb[:], g_b[:, :])
                nc.gpsimd.dma_start(bq_sb[:], bq_d[:, :])
                nc.gpsimd.dma_start(wqkv_sb[:], wqkv.rearrange("(t p) c -> p t c", p=128))
                nc.gpsimd.dma_start(wp2[:], wp_t[:, :])

                # zero regions (j=1 halves of fake pairs, k pad tile)
                nc.gpsimd.memset(q2[:, L:2 * L].bitcast(u32), 0)
                nc.gpsimd.memset(k8[:, 32 * 128:33 * 128].bitcast(u32), 0)
                nc.gpsimd.memset(ones2[:], 1.0)
                nc.gpsimd.memset(eps_sb[:], EPS)

                # ---------- phase A: load x, group stats ----------
                with tc.tile_pool(name="p_x", bufs=1) as p_x, \
                     tc.tile_pool(name="p_st", bufs=1) as p_st, \
                     tc.tile_pool(name="p_gps", bufs=2, space="PSUM") as p_gps:

                    gsum_ps = p_gps.tile([NGROUPS, 512], f32, name="gsum_ps", bufs=1)
                    sqg_ps = p_gps.tile([NGROUPS, 1], f32, name="sqg_ps", bufs=1)
                    # x in 8 transfers of [128, 2048] (2KB lines), h-half
                    # major so group stats can start on the first half.
                    # Pool's queue is busy with weights, so x rides on 3
                    # queues; first-needed tiles first on each queue.
                    x_sched = [(nc.sync, 0, 0), (nc.scalar, 1, 0),
                               (nc.sync, 2, 0), (nc.scalar, 3, 0),
                               (nc.sync, 0, 1), (nc.scalar, 1, 1),
                               (nc.sync, 2, 1), (nc.scalar, 3, 1)]
                    for q, t, h in x_sched:
                        q.dma_start(
                            xn8[:, t, 2048 * h:2048 * (h + 1)],
                            xb8[128 * t:128 * (t + 1),
                                2048 * h:2048 * (h + 1)])

                    # sum(x) per group: fp8 DoubleRow indicator matmuls,
                    # h-half major to chase the DMA.
                    for h in range(2):
                        for i in range(2):
                            for j in range(4 * h, 4 * h + 4):
                                nc.tensor.matmul(gsum_ps[:],
                                                 gt8[:, 2 * i:2 * i + 2, :],
                                                 xn8[:, 2 * i:2 * i + 2,
                                                     512 * j:512 * (j + 1)],
                                                 start=(h == 0 and i == 0 and j == 0),
                                                 stop=(h == 1 and i == 1 and j == 7),
                                                 perf_mode=DR)
                    # sum(x^2) per channel with accum_out, split ACT/DVE
                    acc = p_st.tile([128, 2, 4], f32, name="acc")
                    accs = p_st.tile([128, 4], f32, name="accs")
                    acc16 = p_st.tile([128, 4], bf16, name="acc16")
                    Square = mybir.ActivationFunctionType.Square
                    for t in range(4):
                        for h in range(2):
                            sqscr = p_st.tile([128, 2048], bf16,
                                              name="sqscr", bufs=2)
                            xin = xn8[:, t, 2048 * h:2048 * (h + 1)]
                            if h == 1:
                                nc.scalar.activation(
                                    sqscr[:], xin, Square,
                                    accum_out=acc[:, h, t:t + 1])
                            else:
                                nc.vector.scalar_tensor_tensor(
                                    out=sqscr[:], in0=xin, scalar=1.0,
                                    op0=Alu.mult, in1=xin, op1=Alu.mult,
                                    accum_out=acc[:, h, t:t + 1],
                                )
                    # group-reduce the per-channel sums of squares
                    nc.vector.tensor_add(accs[:], acc[:, 0, :], acc[:, 1, :])
                    nc.vector.tensor_copy(acc16[:], accs[:])
                    for t in range(4):
                        nc.tensor.matmul(sqg_ps[:], gt16[:, t, :],
                                         acc16[:, t:t + 1],
                                         start=(t == 0), stop=(t == 3))

                    sg = p_st.tile([NGROUPS, 2], f32, name="sg")
                    nc.vector.reduce_sum(sg[:, 0:1], gsum_ps[:], axis=mybir.AxisListType.X)
                    nc.vector.tensor_copy(sg[:, 1:2], sqg_ps[:])
                    nc.vector.tensor_scalar_mul(sg[:], sg[:], 1.0 / L)
                    tmpg = p_st.tile([NGROUPS, 1], f32, name="tmpg")
                    nc.vector.tensor_mul(tmpg[:], sg[:, 0:1], sg[:, 0:1])
                    nc.vector.tensor_sub(sg[:, 1:2], sg[:, 1:2], tmpg[:])
                    # rstd = exp(-0.5 * ln(var + eps))
                    nc.scalar.activation(sg[:, 1:2], sg[:, 1:2], Ln, bias=eps_sb[:])
                    nc.scalar.activation(sg[:, 1:2], sg[:, 1:2], Exp, scale=-0.5)

                    # broadcast group stats to channels; per-channel A, B
                    bq_ps = p_gps.tile([128, 1], f32, name="bq_ps", bufs=1)
                    mc_all = p_gps.tile([128, 4, 2], f32, name="mc_all", bufs=1)
                    for t in range(4):
                        nc.tensor.matmul(mc_all[:, t, :], g_sb[:, 128 * t:128 * (t + 1)],
                                         sg[:], start=(t == 0), stop=(t == 3))
                    ab = p_st.tile([128, 4, 2], f32, name="ab")
                    nc.vector.tensor_copy(ab[:], mc_all[:])
                    a_all = p_st.tile([128, 4], f32, name="a_all")
                    b_all = p_st.tile([128, 4], f32, name="b_all")
                    b8a = p_st.tile([128, 4], fp8, name="b8a")
                    nc.vector.tensor_mul(a_all[:], gnwb_sb[:, :, 0], ab[:, :, 1])
                    nc.vector.tensor_mul(b_all[:], ab[:, :, 0], a_all[:])
                    nc.vector.tensor_sub(b_all[:], gnwb_sb[:, :, 1], b_all[:])
                    nc.vector.tensor_copy(b8a[:], b_all[:])
                    nc.gpsimd.dma_start(b_out[:, :], b_all[:])

                    # bias correction Wq@B (unscaled weights); no k bias needed
                    for t in range(4):
                        nc.tensor.matmul(bq_ps[:], wslice(t, 0),
                                         b8a[:, t:t + 1], start=(t == 0), stop=(t == 3))
                    nc.vector.tensor_add(bq2_sb[:], bq_sb[:], bq_ps[:])

                    # fold A into the staged fp8 weights (per-partition scale),
                    # split across ScalarE / DVE to halve the chain latency
                    for t in range(4):
                        if t % 2 == 0:
                            nc.scalar.activation(
                                wqkv_sb[:, t, :], wqkv_sb[:, t, :], Copy,
                                scale=a_all[:, t:t + 1])
                        else:
                            nc.vector.tensor_scalar_mul(
                                out=wqkv_sb[:, t, :], in0=wqkv_sb[:, t, :],
                                scalar1=a_all[:, t:t + 1])

            # ---------- phase D: k chunks 0-2 (no bias), q chunk 0 ----------
            # (k chunks 3..7 and all vT tiles are produced inside chunk 0's
            #  ep loop, overlapped with scores)
            with tc.tile_pool(name="p_dps", bufs=2, space="PSUM") as p_dps:
                for n in range(3):
                    kp = p_dps.tile([128, 512], f32, name="kp")
                    for i in range(2):
                        nc.tensor.matmul(kp[:], wpair(i, 1),
                                         xn8[:, 2 * i:2 * i + 2,
                                             512 * n:512 * (n + 1)],
                                         start=(i == 0), stop=(i == 1),
                                         perf_mode=DR)
                    if n % 2 == 0:
                        nc.vector.tensor_copy(k8[:, 512 * n:512 * (n + 1)], kp[:])
                    else:
                        nc.scalar.copy(k8[:, 512 * n:512 * (n + 1)], kp[:])
                qp = p_dps.tile([128, 512], f32, name="kp")
                for i in range(2):
                    nc.tensor.matmul(qp[:], wpair(i, 0),
                                     xn8[:, 2 * i:2 * i + 2, 0:512],
                                     start=(i == 0), stop=(i == 1),
                                     perf_mode=DR)
                nc.vector.tensor_scalar_add(out=q2[:, 0:512], in0=qp[:],
                                            scalar1=bq2_sb[:])

            # ---------- phase E: attention, software-pipelined by d-chunk ----
            with tc.tile_pool(name="p_est", bufs=2) as p_est, \
                 tc.tile_pool(name="p_y", bufs=2) as p_y, \
                 tc.tile_pool(name="p_scp", bufs=2, space="PSUM") as p_scp, \
                 tc.tile_pool(name="p_oup", bufs=1, space="PSUM") as p_oup, \
                 tc.tile_pool(name="p_yp", bufs=2, space="PSUM") as p_yp:

                def emit_vt_quad(g):
                    # vT e-tiles 4g..4g+3 into one PSUM bank, one cast
                    vp = p_yp.tile([128, 512], f32, name="yp")
                    vp4 = vp.rearrange("p (e c) -> p e c", c=128)
                    for ei in range(4):
                        e = 4 * g + ei
                        for j in range(2):
                            nc.tensor.matmul(vp4[:, ei, :],
                                             xn8[:, 2 * j:2 * j + 2,
                                                 128 * e:128 * (e + 1)],
                                             wpair(j, 2),
                                             start=(j == 0), stop=(j == 1),
                                             perf_mode=DR)
                    if g % 2 == 0:
                        nc.scalar.copy(vt8[:, 512 * g:512 * (g + 1)], vp[:])
                    else:
                        nc.vector.tensor_copy(vt8[:, 512 * g:512 * (g + 1)], vp[:])

                def emit_k_chunk(n):
                    kp = p_yp.tile([128, 512], f32, name="yp")
                    for i in range(2):
                        nc.tensor.matmul(kp[:], wpair(i, 1),
                                         xn8[:, 2 * i:2 * i + 2,
                                             512 * n:512 * (n + 1)],
                                         start=(i == 0), stop=(i == 1),
                                         perf_mode=DR)
                    if n % 2 == 0:
                        nc.vector.tensor_copy(k8[:, 512 * n:512 * (n + 1)], kp[:])
                    else:
                        nc.scalar.copy(k8[:, 512 * n:512 * (n + 1)], kp[:])

                def emit_q_chunk(n):
                    qp = p_yp.tile([128, 512], f32, name="yp")
                    for i in range(2):
                        nc.tensor.matmul(qp[:], wpair(i, 0),
                                         xn8[:, 2 * i:2 * i + 2,
                                             512 * n:512 * (n + 1)],
                                         start=(i == 0), stop=(i == 1),
                                         perf_mode=DR)
                    nc.vector.tensor_scalar_add(
                        out=q2[:, 512 * n:512 * (n + 1)], in0=qp[:],
                        scalar1=bq2_sb[:])

                def emit_tail(dc, ou, zb):
                    # dc's post-attention work: drain ou/zb, proj, y store.
                    # Called from inside emit_chunk(dc+1) after its first
                    # score pair, so the PSUM drain hides under new work.
                    nc.vector.tensor_copy(ou2[:], ou[:])
                    nc.scalar.activation(zsave[0:1, DC * dc:DC * (dc + 1)],
                                         zb[0:1, :], Copy)
                    y4 = p_y.tile([128, 4, C], bf16, name="y4")
                    for j in range(4):
                        yp = p_yp.tile([128, 512], f32, name="yp")
                        nc.tensor.matmul(yp[:], ou2[:, 128 * j:128 * (j + 1)],
                                         wp2[:], start=True, stop=True)
                        if j % 2 == 0:
                            nc.scalar.copy(y4[:, j, :], yp[:])
                        else:
                            nc.vector.tensor_copy(y4[:, j, :], yp[:])
                    r0 = DC * dc
                    if dc == NDC - 1:
                        for j in range(4):
                            eng = (nc.scalar, nc.sync, nc.gpsimd, nc.scalar)[j]
                            eng.dma_start(
                                yt[r0 + 128 * j:r0 + 128 * (j + 1), :],
                                y4[:, j, :])
                    else:
                        eng = nc.gpsimd if dc % 2 == 0 else nc.sync
                        eng.dma_start(
                            yt[r0:r0 + 512, :].rearrange("(j p) o -> p j o", p=128),
                            y4[:])

                def emit_chunk(dc, pending):
                    est = p_est.tile([128, NET * 512], fp8, name="est")
                    est3 = est.rearrange("p (t e) -> p t e", e=512)
                    qrhs = q2v[:, :, dc, :]
                    ou = p_oup.tile([128, 512], f32, name="ou")
                    zb = p_oup.tile([128, 512], f32, name="zb")

                    def av_pair(i):
                        nc.tensor.matmul(ou[:], vt8v[:, 2 * i:2 * i + 2, :],
                                         est3[:, 2 * i:2 * i + 2, :],
                                         start=(i == 0), stop=(i == 15),
                                         perf_mode=DR)

                    def zb_pair(i):
                        nc.tensor.matmul(zb[:], ones2[:],
                                         est3[:, 2 * i:2 * i + 2, :],
                                         start=(i == 0), stop=(i == 15),
                                         perf_mode=DR)

                    for ep in range(16):
                        scAB = p_scp.tile([128, 2, 512], f32, name="scAB")
                        nc.tensor.matmul(scAB[:, 0, :],
                                         k8v[:, 2 * ep:2 * ep + 2, :],
                                         qrhs, start=True, stop=True,
                                         perf_mode=DR)
                        nc.tensor.matmul(scAB[:, 1, :],
                                         k8v[:, 2 * ep + 1:2 * ep + 3, :],
                                         qrhs, start=True, stop=True,
                                         perf_mode=DR)
                        # one 1024-elem exp per pair, ScalarE or DVE
                        pair = est3[:, 2 * ep:2 * ep + 2, :]
                        if ep in ACT_SET:
                            nc.scalar.activation(pair, scAB[:], Exp,
                                                 scale=SCALE)
                        else:
                            nc.vector.tensor_scalar(
                                out=pair.bitcast(i8),
                                in0=scAB[:],
                                scalar1=A_SCH, scalar2=B_SCH,
                                op0=Alu.mult, op1=Alu.add)
                        if ep == 0 and pending is not None:
                            emit_tail(*pending)
                        if dc == 0:
                            if ep < 8:
                                emit_vt_quad(ep)
                            if ep % 2 == 1 and ep < 10:
                                emit_k_chunk(3 + ep // 2)
                        if ep > 0:
                            av_pair(ep - 1)
                            zb_pair(ep - 1)
                    av_pair(15)
                    zb_pair(15)
                    if dc < 7:
                        emit_q_chunk(dc + 1)
                    return (dc, ou, zb)

                pending = None
                for dc in range(NDC):
                    pending = emit_chunk(dc, pending)
                emit_tail(*pending)
                nc.sync.dma_start(zz[:, :], zsave[:, :])

    if split_sync:
        n_w, n_u = _split_multi_sync(nc, mybir)
    return nc


def _prep_inputs(x, gn_w, gn_b, w_qkv, b_qkv, w_proj, b_proj):
    xr = np.ascontiguousarray(np.asarray(x, dtype=np.float32).reshape(NB, C, L))
    w_qkv = np.asarray(w_qkv, dtype=np.float32)
    w_proj = np.asarray(w_proj, dtype=np.float32)
    gn_w = np.asarray(gn_w, dtype=np.float32)
    gn_b = np.asarray(gn_b, dtype=np.float32)
    b_qkv = np.asarray(b_qkv, dtype=np.float32)

    g_ind = np.zeros((NGROUPS, C), dtype=np.float32)
    for g in range(NGROUPS):
        g_ind[g, g * GSIZE:(g + 1) * GSIZE] = 1.0
    gt_m = np.ascontiguousarray(g_ind.T / GSIZE)

    in_maps = []
    for core in range(NCORES):
        bi, h = divmod(core, NH)
        hs = slice(h * HD, (h + 1) * HD)
        xc = np.ascontiguousarray(xr[bi])
        in_maps.append({
            "xb8": xc.astype(FP8),
            "wqkv": np.ascontiguousarray(np.concatenate([
                w_qkv[h * HD:(h + 1) * HD, :].T,
                w_qkv[C + h * HD:C + (h + 1) * HD, :].T,
                w_qkv[2 * C + h * HD:2 * C + (h + 1) * HD, :].T,
            ], axis=1)).astype(FP8),
            "wp_t": np.ascontiguousarray(w_proj[:, hs].T).astype(BF16),
            "bq_d": np.ascontiguousarray(
                b_qkv[h * HD:(h + 1) * HD]).reshape(HD, 1),
            "gnwb": np.ascontiguousarray(np.stack([gn_w, gn_b], axis=1)),
            "g_b": g_ind,
            "gt8_d": gt_m.astype(FP8),
            "gt16_d": gt_m.astype(BF16),
        })
    return xr, in_maps


LAST_RESULTS = None


def kernel(x, gn_w, gn_b, w_qkv, b_qkv, w_proj, b_proj):
    global _NC, LAST_RESULTS
    from concourse.bass_utils import run_bass_kernel_spmd

    if _NC is None:
        _NC = _build_nc()

    xr, in_maps = _prep_inputs(x, gn_w, gn_b, w_qkv, b_qkv, w_proj, b_proj)
    trace = os.environ.get("KBENCH_TRACE", "0") == "1"
    kwargs = {}
    if trace:
        kwargs = dict(trace=True, trace_cores=list(range(NCORES)))
    res = run_bass_kernel_spmd(_NC, in_maps, core_ids=list(range(NCORES)), **kwargs)
    LAST_RESULTS = res

    w_qkv = np.asarray(w_qkv, dtype=np.float32)
    w_proj = np.asarray(w_proj, dtype=np.float32)
    b_qkv = np.asarray(b_qkv, dtype=np.float32)
    b_proj = np.asarray(b_proj, dtype=np.float32)

    out = np.zeros((NB, C, L), dtype=np.float32)
    for core in range(NCORES):
        bi, h = divmod(core, NH)
        r = res.results[core]
        Y = np.asarray(r["yt"], dtype=np.float32)        # [L, C] unnormalized y^T
        Z = np.asarray(r["zz"], dtype=np.float32).reshape(L)
        B = np.asarray(r["b_out"], dtype=np.float32).T.reshape(C)
        wv = w_qkv[2 * C + h * HD:2 * C + (h + 1) * HD, :]   # [128, 512]
        bv = b_qkv[2 * C + h * HD:2 * C + (h + 1) * HD] + wv @ B
        wpbv = w_proj[:, h * HD:(h + 1) * HD] @ bv       # [C]
        out[bi] += (Y / Z[:, None] + wpbv[None, :]).T
    out += b_proj[None, :, None]
    out += xr
    return out.reshape(NB, C, 64, 64).astype(np.float32)
